# revision 17
# baseline (speedup 1.0000x reference)
"""Trainium2 Bass kernel for nn_H3TCSNetwork (dense MLP, 8-core data parallel).

Network (per row of x[B,7]):
  f = silu(x@W0+b0); f = silu(f@W1+b1); f = silu(f@W2+b2); f = silu(f@W3+b3)
  heads = (f@Wh + bh)                      # [B, 2695] = [B, 77*35]
  out   = heads * scale[b, head_group]     # per-head profile scaling

Sharding: batch split across 8 cores (8192 rows each), weights replicated.

On-chip layout:
  * Layers 0-3 run feature-major: activations are [feat(part), batch(free)],
    weights are the stationary operand.  Bias+silu fuse into one ACT op per
    PSUM tile (bias is per-partition there).
  * Head runs batch-major: f4 tiles [feat, batch128] are the stationary
    operand, Wh is moving, so PSUM comes out [batch(part), cols(free)] and the
    output DMA writes 2KB contiguous per partition.
  * Head epilogue on DVE: out = psum + bh_rep (bias replicated across
    partitions), then in-place per-partition multiply by the profile for the
    left/right/neck column ranges (local heads have scale 1).
  * Profiles (sigmoid/exp of lam=x[:,0]) computed on ACT in batch-major
    [128, 64] layout; column t is batch-tile t's per-partition scalar.
"""

import os
import sys

import numpy as np

sys.path.insert(0, "/opt/trn_rl_repo")

import concourse.bass as bass
import concourse.mybir as mybir
from concourse import bacc
from concourse.tile import TileContext

N_CORES = 8
B = 65536
BC = B // N_CORES  # rows per core
HID = 256
NH = 2695  # 77 heads * 35 outs
NHP = 2696  # padded: fp32r matmul needs an even moving dim (and >=256 is faster)
CH = 512  # batch chunk (matmul moving free dim)
NCHUNK = BC // CH  # 16
NBT = BC // 128  # 64 batch tiles per core
COL_CHUNKS = [(0, 512), (512, 1024), (1024, 1536), (1536, 2048), (2048, 2372), (2372, NHP)]
# head column ranges scaled by a profile (local heads 0:1225 have scale 1)
GROUPS = [(1225, 1715, "left"), (1715, 2205, "right"), (2205, NH, "neck")]

F32 = mybir.dt.float32
# matmul operand dtype: float32 (exact, 4 cyc/row), float32r (1 cyc/row N>=256),
# float16 (1 cyc/row, ~1e-3 accuracy)
MM_DT = getattr(mybir.dt, os.environ.get("MM_DT", "float32r"))
AF = mybir.ActivationFunctionType
ALU = mybir.AluOpType


def build_nc() -> bass.Bass:
    nc = bacc.Bacc()

    x = nc.dram_tensor("x", [BC, 7], F32, kind="ExternalInput")
    W0 = nc.dram_tensor("W0", [7, HID], F32, kind="ExternalInput")
    b0 = nc.dram_tensor("b0", [HID], F32, kind="ExternalInput")
    W1 = nc.dram_tensor("W1", [HID, HID], F32, kind="ExternalInput")
    b1 = nc.dram_tensor("b1", [HID], F32, kind="ExternalInput")
    W2 = nc.dram_tensor("W2", [HID, HID], F32, kind="ExternalInput")
    b2 = nc.dram_tensor("b2", [HID], F32, kind="ExternalInput")
    W3 = nc.dram_tensor("W3", [HID, HID], F32, kind="ExternalInput")
    b3 = nc.dram_tensor("b3", [HID], F32, kind="ExternalInput")
    Wh = nc.dram_tensor("Wh", [HID, NHP], F32, kind="ExternalInput")
    bh = nc.dram_tensor("bh", [NHP], F32, kind="ExternalInput")
    out = nc.dram_tensor("out", [BC, NH], F32, kind="ExternalOutput")

    # fp32r/fp16 operands must be produced by compute ops (HW rounds on write);
    # a raw DMA of fp32 bits fails BIR verification for fp32r consumers.
    cast = MM_DT != F32

    with TileContext(nc) as tc:
        with (
            tc.tile_pool(name="wpool", bufs=1) as wpool,
            tc.tile_pool(name="stage", bufs=2) as stage,
            tc.tile_pool(name="xpool", bufs=3) as xpool,
            tc.tile_pool(name="fpool", bufs=3) as fpool,
            tc.tile_pool(name="opool", bufs=8) as opool,
            tc.tile_pool(name="pl", bufs=4, space="PSUM") as pl,
            tc.tile_pool(name="ph", bufs=4, space="PSUM") as ph,
        ):
            # ---------------- weights + biases ----------------
            def load_w(dram, shape, nm):
                """DRAM fp32 -> SBUF tile(s) in MM_DT, split into <=128-row k-tiles."""
                rows, cols = shape
                ks = []
                for k in range(0, rows, 128):
                    p = min(128, rows - k)
                    t32 = stage.tile([p, cols], F32, name=f"{nm}s{k}", tag="wstage") if cast else \
                        wpool.tile([p, cols], F32, name=f"{nm}_{k}")
                    nc.sync.dma_start(out=t32[:, :], in_=dram[k : k + p, :])
                    if cast:
                        t16 = wpool.tile([p, cols], MM_DT, name=f"{nm}_{k}")
                        nc.vector.tensor_copy(out=t16[:, :], in_=t32[:, :])
                        ks.append(t16)
                    else:
                        ks.append(t32)
                return ks

            w0 = load_w(W0, (7, HID), "w0")
            w1 = load_w(W1, (HID, HID), "w1")
            w2 = load_w(W2, (HID, HID), "w2")
            w3 = load_w(W3, (HID, HID), "w3")
            wh = load_w(Wh, (HID, NHP), "wh")

            # biases b0..b3 as [128, 2] (col m = m-tile), per-partition slices
            bts = []
            for nm, bd in (("b0", b0), ("b1", b1), ("b2", b2), ("b3", b3)):
                bt = wpool.tile([128, 2], F32, name=f"{nm}t")
                nc.sync.dma_start(out=bt[:, :], in_=bd.rearrange("(m p) -> p m", p=128))
                bts.append(bt)

            # bh replicated across all 128 partitions
            bh_rep = wpool.tile([128, NHP], F32, name="bh_rep")
            nc.sync.dma_start(out=bh_rep[:, :], in_=bh[:].partition_broadcast(128))

            # ---------------- profiles (batch-major [128, NBT]) ----------------
            lam = wpool.tile([128, NBT], F32, name="lam")
            nc.sync.dma_start(
                out=lam[:, :], in_=x[:, 0:1].rearrange("(t p) o -> p (t o)", p=128)
            )
            s_t = wpool.tile([128, NBT], F32, name="s_t")  # sigmoid -> "right"
            left_t = wpool.tile([128, NBT], F32, name="left_t")
            sq_t = wpool.tile([128, NBT], F32, name="sq_t")
            neck_t = wpool.tile([128, NBT], F32, name="neck_t")
            nc.scalar.activation(s_t[:, :], lam[:, :], AF.Sigmoid, scale=5.0 / 0.15)
            nc.scalar.activation(left_t[:, :], s_t[:, :], AF.Copy, bias=1.0, scale=-1.0)
            nc.scalar.activation(sq_t[:, :], lam[:, :], AF.Square, scale=5.0)
            nc.scalar.activation(neck_t[:, :], sq_t[:, :], AF.Exp, scale=-1.0)
            prof = {"left": left_t, "right": s_t, "neck": neck_t}

            xT = x.rearrange("b k -> k b")
            layer_ws = [(w0, bts[0]), (w1, bts[1]), (w2, bts[2]), (w3, bts[3])]

            for c in range(NCHUNK):
                # -------- x chunk, transposed to [7, CH] --------
                xt32 = xpool.tile([7, CH], F32, name=f"xt32_{c}", tag="xt32")
                nc.sync.dma_start(out=xt32[:, :], in_=xT[:, c * CH : (c + 1) * CH])
                if cast:
                    xt = xpool.tile([7, CH], MM_DT, name=f"xt_{c}", tag="xt")
                    nc.vector.tensor_copy(out=xt[:, :], in_=xt32[:, :])
                else:
                    xt = xt32

                # -------- layers 0..3, feature-major --------
                fprev = [xt]
                for L, (wk, btile) in enumerate(layer_ws):
                    fout = []
                    for m in range(2):
                        ps = pl.tile([128, CH], F32, name=f"pl{c}_{L}_{m}", tag="pl")
                        nk = len(fprev)
                        for k in range(nk):
                            nc.tensor.matmul(
                                ps[:, :],
                                lhsT=wk[k][:, m * 128 : (m + 1) * 128],
                                rhs=fprev[k][:, :],
                                start=(k == 0),
                                stop=(k == nk - 1),
                            )
                        ft = fpool.tile(
                            [128, CH], MM_DT if cast else F32,
                            name=f"f{c}_{L}_{m}", tag=f"f{m}",
                        )
                        nc.scalar.activation(
                            ft[:, :], ps[:, :], AF.Silu, bias=btile[:, m : m + 1]
                        )
                        fout.append(ft)
                    fprev = fout

                # -------- head, batch-major, 4 batch-tiles of 128 --------
                for bt in range(4):
                    gt = c * 4 + bt
                    for n0, n1 in COL_CHUNKS:
                        w = n1 - n0  # matmul width (padded)
                        wo = min(n1, NH) - n0  # output width (unpadded)
                        psh = ph.tile([128, 512], F32, name=f"ph{gt}_{n0}", tag="ph")
                        for k in range(2):
                            nc.tensor.matmul(
                                psh[:, :w],
                                lhsT=fprev[k][:, bt * 128 : (bt + 1) * 128],
                                rhs=wh[k][:, n0:n1],
                                start=(k == 0),
                                stop=(k == 1),
                            )
                        ot = opool.tile([128, 512], F32, name=f"ot{gt}_{n0}", tag="ot")
                        nc.vector.tensor_add(
                            ot[:, :wo], psh[:, :wo], bh_rep[:, n0 : n0 + wo]
                        )
                        for g0, g1, pname in GROUPS:
                            lo, hi = max(g0, n0), min(g1, n0 + wo)
                            if lo < hi:
                                nc.vector.tensor_scalar_mul(
                                    ot[:, lo - n0 : hi - n0],
                                    ot[:, lo - n0 : hi - n0],
                                    prof[pname][:, gt : gt + 1],
                                )
                        nc.sync.dma_start(
                            out=out[gt * 128 : (gt + 1) * 128, n0 : n0 + wo],
                            in_=ot[:, :wo],
                        )
    nc.compile()
    return nc


_NC = None


def make_in_maps(inputs):
    x = np.ascontiguousarray(inputs["x"], dtype=np.float32)
    shared = {
        k: np.ascontiguousarray(inputs[k], dtype=np.float32)
        for k in ("W0", "b0", "W1", "b1", "W2", "b2", "W3", "b3")
    }
    shared["Wh"] = np.ascontiguousarray(
        np.pad(inputs["Wh"].astype(np.float32), ((0, 0), (0, NHP - NH)))
    )
    shared["bh"] = np.ascontiguousarray(
        np.pad(inputs["bh"].astype(np.float32), (0, NHP - NH))
    )
    return [{"x": x[c * BC : (c + 1) * BC], **shared} for c in range(N_CORES)]


def kernel(**inputs: np.ndarray) -> np.ndarray:
    global _NC
    from concourse.bass_utils import run_bass_kernel_spmd

    if _NC is None:
        _NC = build_nc()

    in_maps = make_in_maps(inputs)
    res = run_bass_kernel_spmd(_NC, in_maps, core_ids=list(range(N_CORES)))
    full = np.concatenate([res.results[c]["out"] for c in range(N_CORES)], axis=0)
    return full.reshape(B, 77, 35)


# revision 19
# speedup vs baseline: 1.5031x; 1.5031x over previous
"""Trainium2 Bass kernel for nn_H3TCSNetwork (dense MLP, 8-core data parallel).

Network (per row of x[B,7]):
  f = silu(x@W0+b0); f = silu(f@W1+b1); f = silu(f@W2+b2); f = silu(f@W3+b3)
  heads = (f@Wh + bh)                      # [B, 2695] = [B, 77*35]
  out   = heads * scale[b, head_group]     # per-head profile scaling

Sharding: batch split across 8 cores (8192 rows each), weights replicated.

On-chip layout:
  * Layers 0-3 run feature-major: activations are [feat(part), batch(free)],
    weights are the stationary operand.  Bias+silu fuse into one ACT op per
    PSUM tile (bias is per-partition there).
  * Head runs batch-major: f4 tiles [feat, batch128] are the stationary
    operand, Wh is moving, so PSUM comes out [batch(part), cols(free)] and the
    output DMA writes 2KB contiguous per partition.
  * Head epilogue on DVE: out = psum + bh_rep (bias replicated across
    partitions), then in-place per-partition multiply by the profile for the
    left/right/neck column ranges (local heads have scale 1).
  * Profiles (sigmoid/exp of lam=x[:,0]) computed on ACT in batch-major
    [128, 64] layout; column t is batch-tile t's per-partition scalar.
"""

import os
import sys

import numpy as np

sys.path.insert(0, "/opt/trn_rl_repo")

import concourse.bass as bass
import concourse.mybir as mybir
from concourse import bacc
from concourse.tile import TileContext

N_CORES = 8
B = 65536
BC = B // N_CORES  # rows per core
HID = 256
NH = 2695  # 77 heads * 35 outs
NHP = 2696  # padded: fp32r matmul needs an even moving dim (and >=256 is faster)
CH = 512  # batch chunk (matmul moving free dim)
NCHUNK = BC // CH  # 16
NBT = BC // 128  # 64 batch tiles per core
COL_CHUNKS = [(0, 512), (512, 1024), (1024, 1536), (1536, 2048), (2048, 2372), (2372, NHP)]
# head column ranges scaled by a profile (local heads 0:1225 have scale 1)
GROUPS = [(1225, 1715, "left"), (1715, 2205, "right"), (2205, NH, "neck")]

F32 = mybir.dt.float32
# matmul operand dtype: float32 (exact, 4 cyc/row), float32r (1 cyc/row N>=256),
# float16 (1 cyc/row, ~1e-3 accuracy)
MM_DT = getattr(mybir.dt, os.environ.get("MM_DT", "float32r"))
AF = mybir.ActivationFunctionType
ALU = mybir.AluOpType


def build_nc() -> bass.Bass:
    nc = bacc.Bacc()

    x = nc.dram_tensor("x", [BC, 7], F32, kind="ExternalInput")
    W0 = nc.dram_tensor("W0", [7, HID], F32, kind="ExternalInput")
    b0 = nc.dram_tensor("b0", [HID], F32, kind="ExternalInput")
    W1 = nc.dram_tensor("W1", [HID, HID], F32, kind="ExternalInput")
    b1 = nc.dram_tensor("b1", [HID], F32, kind="ExternalInput")
    W2 = nc.dram_tensor("W2", [HID, HID], F32, kind="ExternalInput")
    b2 = nc.dram_tensor("b2", [HID], F32, kind="ExternalInput")
    W3 = nc.dram_tensor("W3", [HID, HID], F32, kind="ExternalInput")
    b3 = nc.dram_tensor("b3", [HID], F32, kind="ExternalInput")
    Wh = nc.dram_tensor("Wh", [HID, NHP], F32, kind="ExternalInput")
    bh = nc.dram_tensor("bh", [NHP], F32, kind="ExternalInput")
    out = nc.dram_tensor("out", [BC, NH], F32, kind="ExternalOutput")

    # fp32r/fp16 operands must be produced by compute ops (HW rounds on write);
    # a raw DMA of fp32 bits fails BIR verification for fp32r consumers.
    cast = MM_DT != F32

    with TileContext(nc) as tc:
        with (
            tc.tile_pool(name="wpool", bufs=1) as wpool,
            tc.tile_pool(name="stage", bufs=2) as stage,
            tc.tile_pool(name="xpool", bufs=3) as xpool,
            tc.tile_pool(name="fpool", bufs=3) as fpool,
            tc.tile_pool(name="opool", bufs=3) as opool,
            tc.tile_pool(name="pl", bufs=4, space="PSUM") as pl,
            tc.tile_pool(name="ph", bufs=4, space="PSUM") as ph,
        ):
            # ---------------- weights + biases ----------------
            def load_w(dram, shape, nm):
                """DRAM fp32 -> SBUF tile(s) in MM_DT, split into <=128-row k-tiles."""
                rows, cols = shape
                ks = []
                for k in range(0, rows, 128):
                    p = min(128, rows - k)
                    t32 = stage.tile([p, cols], F32, name=f"{nm}s{k}", tag="wstage") if cast else \
                        wpool.tile([p, cols], F32, name=f"{nm}_{k}")
                    nc.sync.dma_start(out=t32[:, :], in_=dram[k : k + p, :])
                    if cast:
                        t16 = wpool.tile([p, cols], MM_DT, name=f"{nm}_{k}")
                        nc.vector.tensor_copy(out=t16[:, :], in_=t32[:, :])
                        ks.append(t16)
                    else:
                        ks.append(t32)
                return ks

            w0 = load_w(W0, (7, HID), "w0")
            w1 = load_w(W1, (HID, HID), "w1")
            w2 = load_w(W2, (HID, HID), "w2")
            w3 = load_w(W3, (HID, HID), "w3")
            wh = load_w(Wh, (HID, NHP), "wh")

            # biases b0..b3 as [128, 2] (col m = m-tile), per-partition slices
            bts = []
            for nm, bd in (("b0", b0), ("b1", b1), ("b2", b2), ("b3", b3)):
                bt = wpool.tile([128, 2], F32, name=f"{nm}t")
                nc.sync.dma_start(out=bt[:, :], in_=bd.rearrange("(m p) -> p m", p=128))
                bts.append(bt)

            # bh replicated across all 128 partitions
            bh_rep = wpool.tile([128, NHP], F32, name="bh_rep")
            nc.sync.dma_start(out=bh_rep[:, :], in_=bh[:].partition_broadcast(128))

            # ---------------- profiles (batch-major [128, NBT]) ----------------
            lam = wpool.tile([128, NBT], F32, name="lam")
            nc.sync.dma_start(
                out=lam[:, :], in_=x[:, 0:1].rearrange("(t p) o -> p (t o)", p=128)
            )
            s_t = wpool.tile([128, NBT], F32, name="s_t")  # sigmoid -> "right"
            left_t = wpool.tile([128, NBT], F32, name="left_t")
            sq_t = wpool.tile([128, NBT], F32, name="sq_t")
            neck_t = wpool.tile([128, NBT], F32, name="neck_t")
            nc.scalar.activation(s_t[:, :], lam[:, :], AF.Sigmoid, scale=5.0 / 0.15)
            nc.scalar.activation(left_t[:, :], s_t[:, :], AF.Copy, bias=1.0, scale=-1.0)
            nc.scalar.activation(sq_t[:, :], lam[:, :], AF.Square, scale=5.0)
            nc.scalar.activation(neck_t[:, :], sq_t[:, :], AF.Exp, scale=-1.0)
            prof = {"left": left_t, "right": s_t, "neck": neck_t}

            xT = x.rearrange("b k -> k b")
            layer_ws = [(w0, bts[0]), (w1, bts[1]), (w2, bts[2]), (w3, bts[3])]

            for c in range(NCHUNK):
                # -------- x chunk, transposed to [7, CH] --------
                xt32 = xpool.tile([7, CH], F32, name=f"xt32_{c}", tag="xt32")
                nc.sync.dma_start(out=xt32[:, :], in_=xT[:, c * CH : (c + 1) * CH])
                if cast:
                    xt = xpool.tile([7, CH], MM_DT, name=f"xt_{c}", tag="xt")
                    nc.vector.tensor_copy(out=xt[:, :], in_=xt32[:, :])
                else:
                    xt = xt32

                # -------- layers 0..3, feature-major --------
                fprev = [xt]
                for L, (wk, btile) in enumerate(layer_ws):
                    fout = []
                    for m in range(2):
                        ps = pl.tile([128, CH], F32, name=f"pl{c}_{L}_{m}", tag="pl")
                        nk = len(fprev)
                        for k in range(nk):
                            nc.tensor.matmul(
                                ps[:, :],
                                lhsT=wk[k][:, m * 128 : (m + 1) * 128],
                                rhs=fprev[k][:, :],
                                start=(k == 0),
                                stop=(k == nk - 1),
                            )
                        ft = fpool.tile(
                            [128, CH], MM_DT if cast else F32,
                            name=f"f{c}_{L}_{m}", tag=f"f{m}",
                        )
                        nc.scalar.activation(
                            ft[:, :], ps[:, :], AF.Silu, bias=btile[:, m : m + 1]
                        )
                        fout.append(ft)
                    fprev = fout

                # -------- head, batch-major, 4 batch-tiles of 128 --------
                for bt in range(4):
                    gt = c * 4 + bt
                    # full output row block; one big DMA at the end
                    ot = opool.tile([128, NH], F32, name=f"ot{gt}", tag="ot")
                    for n0, n1 in COL_CHUNKS:
                        w = n1 - n0  # matmul width (padded)
                        wo = min(n1, NH) - n0  # output width (unpadded)
                        psh = ph.tile([128, 512], F32, name=f"ph{gt}_{n0}", tag="ph")
                        for k in range(2):
                            nc.tensor.matmul(
                                psh[:, :w],
                                lhsT=fprev[k][:, bt * 128 : (bt + 1) * 128],
                                rhs=wh[k][:, n0:n1],
                                start=(k == 0),
                                stop=(k == 1),
                            )
                        # local (scale=1) sub-range: out = psum + bh  (DVE)
                        lo, hi = n0, min(n0 + wo, GROUPS[0][0])
                        if lo < hi:
                            nc.vector.tensor_add(
                                ot[:, lo:hi],
                                psh[:, lo - n0 : hi - n0],
                                bh_rep[:, lo:hi],
                            )
                        # scaled sub-ranges: out = psum*prof (ACT)
                        #                    out = bh*prof + out (DVE, one pass)
                        for g0, g1, pname in GROUPS:
                            lo, hi = max(g0, n0), min(g1, n0 + wo)
                            if lo < hi:
                                p_ap = prof[pname][:, gt : gt + 1]
                                nc.scalar.activation(
                                    ot[:, lo:hi],
                                    psh[:, lo - n0 : hi - n0],
                                    AF.Copy,
                                    scale=p_ap,
                                )
                                nc.vector.scalar_tensor_tensor(
                                    ot[:, lo:hi],
                                    bh_rep[:, lo:hi],
                                    p_ap,
                                    ot[:, lo:hi],
                                    ALU.mult,
                                    ALU.add,
                                )
                    # one 1.38MB DMA per row block, cycling HWDGE queues
                    eng = (nc.sync, nc.gpsimd, nc.scalar)[gt % 3]
                    eng.dma_start(
                        out=out[gt * 128 : (gt + 1) * 128, :], in_=ot[:, :NH]
                    )
    nc.compile()
    return nc


_NC = None


def make_in_maps(inputs):
    x = np.ascontiguousarray(inputs["x"], dtype=np.float32)
    shared = {
        k: np.ascontiguousarray(inputs[k], dtype=np.float32)
        for k in ("W0", "b0", "W1", "b1", "W2", "b2", "W3", "b3")
    }
    shared["Wh"] = np.ascontiguousarray(
        np.pad(inputs["Wh"].astype(np.float32), ((0, 0), (0, NHP - NH)))
    )
    shared["bh"] = np.ascontiguousarray(
        np.pad(inputs["bh"].astype(np.float32), (0, NHP - NH))
    )
    return [{"x": x[c * BC : (c + 1) * BC], **shared} for c in range(N_CORES)]


def kernel(**inputs: np.ndarray) -> np.ndarray:
    global _NC
    from concourse.bass_utils import run_bass_kernel_spmd

    if _NC is None:
        _NC = build_nc()

    in_maps = make_in_maps(inputs)
    res = run_bass_kernel_spmd(_NC, in_maps, core_ids=list(range(N_CORES)))
    full = np.concatenate([res.results[c]["out"] for c in range(N_CORES)], axis=0)
    return full.reshape(B, 77, 35)
